# revision 1
# baseline (speedup 1.0000x reference)
"""Cross-attention Trainium2 Bass kernel (bf16, software-pipelined).

Sharding: data-parallel over batch — 16 batches across 8 cores, 2 per core.
Weights replicated. Each core computes its 2 batches fully; no collectives.

All matmuls run in bf16 (1 cycle/row at any moving size on TRN2's PE).
PE transposes are eliminated entirely: x and attn are transposed by the
DMA crossbar (dma_start_transpose, 2-byte dtype, 16x128 xbar tiles), whose
destination mapping is out[p, c, f] = in[f, c*128 + p] (verified on hw).

Per 512-row x tile:
  x_bf   = bf16(x tile)                 (SWDGE casting DMA, Pool engine)
  xT     = DMA-transpose(x_bf)          -> [d, s] layout
  qT     = Wq^T @ xT                    (PE; PSUM->SBUF copy on DVE)
  per head h:  scT = kT_h^T @ qT_h      [77, 512] (PE)
               et_h = exp(0.125 * scT)  (ACT, bf16 out)
  per s-chunk c (128 rows), per 4-head group:
    pa[:, hh, :] = et_h_chunk^T @ [v_h | 1 | 1]   (PE; col 64 = softmax denom)
    rr = 1/pa[:, :, 64]                 (DVE)
    attn_n = pa[:, :, 0:64] * rr        (DVE, bf16, per-partition scalar)
  attnT  = DMA-transpose(attn_n)        -> [e, s] layout
  out    = attnT^T @ Wout + bout        (PE; bias added during the
           PSUM->SBUF move by DVE tensor_add; SWDGE store)

The per-engine instruction streams are software-pipelined with a 2-tile
lag so no engine waits on same-tile producers:
  iteration k (PE order): qT(k) | scores(k) | attnU(k-1) | outproj(k-2)
with x loads / DMA transposes issued 1-2 iterations ahead.

Weights are cast-loaded fp32->bf16 by SWDGE DMA, laid out
"(c p) e -> p c e" so partition p of chunk c holds row c*128+p, matching
the DMA-transpose output mapping. TRN2 allows 1 semaphore wait per
instruction — generate_event_semaphores() legalizes multi-wait
instructions that Tile emits.
"""

import numpy as np

import bass_rust as _bass_rust
import concourse.bass as bass
import concourse.mybir as mybir
import concourse.tile as tile
from concourse.bass import broadcast_tensor_aps
from concourse.bass_utils import run_bass_kernel_spmd

N_CORES = 8
B, SQ, DM = 16, 4096, 512
SKV, DC = 77, 768
H, DH = 8, 64
INNER = 512
BPC = B // N_CORES  # batches per core
NT = SQ // 512      # x tiles per batch
NTILES = BPC * NT   # total x tiles per core

F32 = mybir.dt.float32
BF16 = mybir.dt.bfloat16

AF = mybir.ActivationFunctionType


QSPLIT = 2


def build_nc(trace_sim=False, nbig=3, nsc=3, nau=2, nqt=2, net=2, nan=2, nat=2, nos=2, nkv=2, nxl=3):
    nc = bass.Bass()

    x_d = nc.dram_tensor("x", [BPC, SQ, DM], F32, kind="ExternalInput")
    ctx_d = nc.dram_tensor("context", [BPC, SKV, DC], F32, kind="ExternalInput")
    wq_d = nc.dram_tensor("Wq", [DM, INNER], F32, kind="ExternalInput")
    wk_d = nc.dram_tensor("Wk", [DC, INNER], F32, kind="ExternalInput")
    wv_d = nc.dram_tensor("Wv", [DC, INNER], F32, kind="ExternalInput")
    wo_d = nc.dram_tensor("Wout", [INNER, INNER], F32, kind="ExternalInput")
    bo_d = nc.dram_tensor("bout", [INNER], F32, kind="ExternalInput")
    out_d = nc.dram_tensor("out", [BPC, SQ, DM], F32, kind="ExternalOutput")

    with tile.TileContext(nc, trace_sim=trace_sim) as tc:
        with (
            tc.tile_pool(name="const", bufs=1) as consts,
            tc.tile_pool(name="kvp", bufs=nkv) as kvp,
            tc.tile_pool(name="xload", bufs=nxl) as xload,
            tc.tile_pool(name="xtp", bufs=3) as xtp,
            tc.tile_pool(name="qtp", bufs=nqt) as qtp,
            tc.tile_pool(name="etp", bufs=net) as etp,
            tc.tile_pool(name="rrp", bufs=8) as rrp,
            tc.tile_pool(name="anp", bufs=nan) as anp,
            tc.tile_pool(name="atp", bufs=nat) as atp,
            tc.tile_pool(name="osp", bufs=nos) as osp,
            tc.tile_pool(name="pbig", bufs=nbig, space="PSUM") as pbig,
            tc.tile_pool(name="psc", bufs=nsc, space="PSUM") as psc,
            tc.tile_pool(name="pau", bufs=nau, space="PSUM") as pau,
        ):
            # ---- weights: casting SWDGE loads, fp32 DRAM -> bf16 SBUF ----
            # layout "(c p) e -> p c e": partition p of chunk c holds row
            # c*128+p — same mapping as the DMA-transpose destination.
            # Declared here; loads are emitted below in DMA service order so
            # what the kv phase and the first tiles need lands first.
            wk_sb = consts.tile([128, DC // 128, INNER], BF16, tag="wk")
            wv_sb = consts.tile([128, DC // 128, INNER], BF16, tag="wv")
            wq_sb = consts.tile([128, DM // 128, INNER], BF16, tag="wq")
            wo_sb = consts.tile([128, INNER // 128, INNER], BF16, tag="wo")
            bias_b = consts.tile([128, INNER], F32, tag="bias")

            def emit_ctx(b):
                # ctx cast-load into a 80-partition tile (pad rows 77..79 are
                # never read downstream; DMA-T needs p % 16 == 0)
                ctx_bf = kvp.tile([80, DC], BF16, tag="ctx")
                nc.gpsimd.dma_start(out=ctx_bf[0:SKV, :], in_=ctx_d[b])
                ctxT = kvp.tile([128, DC // 128, 80], BF16, tag="ctxT")
                nc.sync.dma_start_transpose(out=ctxT, in_=ctx_bf[:, :])
                return ctxT

            def emit_kT(ctxT):
                # kT[e, kv]: lhsT = Wk chunk, rhs = ctxT chunk
                kT_sb = kvp.tile([128, INNER // 128, SKV], BF16, tag="kT")
                for i in range(INNER // 128):
                    pk = pbig.tile([128, 512], F32, tag="big")
                    for j in range(DC // 128):
                        nc.tensor.matmul(
                            out=pk[:, 0:SKV],
                            lhsT=wk_sb[:, j, i * 128:(i + 1) * 128],
                            rhs=ctxT[:, j, 0:SKV],
                            start=(j == 0), stop=(j == DC // 128 - 1),
                        )
                    nc.scalar.copy(out=kT_sb[:, i, :], in_=pk[:, 0:SKV])
                return kT_sb

            def emit_v(ctxT):
                # v computed transposed ([e, kv], full-width matmuls), then
                # DMA-transposed back to [kv, e]; only kv rows 0..76 of the
                # transpose output are ever read, so pad columns stay garbage
                vT_sb = kvp.tile([128, INNER // 128, 128], BF16, tag="vT")
                for i in range(INNER // 128):
                    pv = pbig.tile([128, 512], F32, tag="big")
                    for j in range(DC // 128):
                        nc.tensor.matmul(
                            out=pv[:, 0:SKV],
                            lhsT=wv_sb[:, j, i * 128:(i + 1) * 128],
                            rhs=ctxT[:, j, 0:SKV],
                            start=(j == 0), stop=(j == DC // 128 - 1),
                        )
                    nc.scalar.copy(out=vT_sb[:, i, 0:SKV], in_=pv[:, 0:SKV])
                v_kv = kvp.tile([128, INNER // 128, 128], BF16, tag="v_kv")
                nc.sync.dma_start_transpose(out=v_kv, in_=vT_sb[:, :, :])

                # v_aug[kv, h, 0:64] = v_h, col 64 = 1 (softmax denominator)
                v_aug = kvp.tile([SKV, H, 66], BF16, tag="v_aug")
                nc.scalar.copy(
                    out=v_aug[:, :, 0:64],
                    in_=v_kv[0:SKV, :, :].rearrange("p a b -> p (a b)").rearrange(
                        "p (h d) -> p h d", h=H
                    ),
                )
                nc.vector.memset(v_aug[:, :, 64:66], 1.0)
                return v_aug

            def emit_kv(ctxT):
                kT_sb = emit_kT(ctxT)
                v_aug = emit_v(ctxT)
                return kT_sb, v_aug

            # per-tile stage emitters; state[k] carries live tiles of tile k
            def bs(k):
                return k // NT, (k % NT) * 512

            def emit_xload(k):
                b, s0 = bs(k)
                x_bf = xload.tile([128, 4, DM], BF16, tag="x")
                nc.gpsimd.dma_start(
                    out=x_bf,
                    in_=x_d[b, s0:s0 + 512, :].rearrange("(t p) d -> p t d", p=128),
                )
                return x_bf

            def emit_xT(x_bf):
                # one merged transpose: in [128, 2048] -> out row r = c*128+p
                # lands as xT[p, t, c, ss] = x[t*128+ss, c*128+p]
                xT = xtp.tile([128, 4, 4, 128], BF16, tag="xT")
                nc.sync.dma_start_transpose(out=xT, in_=x_bf[:, :, :])
                return xT

            def emit_qT(xT, all_dve=False, all_act=False):
                qT = qtp.tile([128, 4, 512], BF16, tag="qT")
                for i in range(4):
                    pq = pbig.tile([128, 512], F32, tag="big")
                    for j in range(4):
                        nc.tensor.matmul(
                            out=pq,
                            lhsT=wq_sb[:, j, i * 128:(i + 1) * 128],
                            rhs=xT[:, :, j, :],
                            start=(j == 0), stop=(j == 3),
                        )
                    if (i < QSPLIT or all_dve) and not all_act:
                        nc.vector.tensor_copy(qT[:, i, :], pq)
                    else:
                        nc.scalar.copy(out=qT[:, i, :], in_=pq)
                return qT

            def emit_scores(qT, kT_sb):
                et = etp.tile([SKV, H, 512], BF16, tag="et")
                for h in range(H):
                    i, r0 = h // 2, (h % 2) * 64
                    ps = psc.tile([SKV, 512], F32, tag="sc")
                    nc.tensor.matmul(
                        out=ps,
                        lhsT=kT_sb[r0:r0 + 64, i, :],
                        rhs=qT[r0:r0 + 64, i, :],
                        start=True, stop=True,
                    )
                    nc.scalar.activation(
                        out=et[:, h, :], in_=ps, func=AF.Exp, scale=0.125,
                    )
                return et

            def emit_attn(et, v_aug):
                attn_n = anp.tile([128, 4, 512], BF16, tag="attn_n")
                attnT = atp.tile([128, 4, 4, 128], BF16, tag="attnT")
                for c in range(4):
                    for g in range(2):
                        pa = pau.tile([128, 4, 66], F32, tag="attnU")
                        for hh in range(4):
                            h = g * 4 + hh
                            nc.tensor.matmul(
                                out=pa[:, hh, :],
                                lhsT=et[:, h, c * 128:(c + 1) * 128],
                                rhs=v_aug[:, h, :],
                                start=True, stop=True,
                            )
                        rr = rrp.tile([128, 4, 1], F32, tag="rr")
                        nc.vector.reciprocal(out=rr, in_=pa[:, :, 64:65])
                        out_ap = attn_n[:, c, g * 256:(g + 1) * 256].rearrange(
                            "p (h d) -> p h d", h=4
                        )
                        in0, in1 = broadcast_tensor_aps(pa[:, :, 0:64], rr)
                        nc.vector.tensor_mul(out_ap, in0, in1)
                    nc.sync.dma_start_transpose(
                        out=attnT[:, c, :, :], in_=attn_n[:, c, :],
                    )
                return attnT

            def emit_outproj(attnT, k, last=False):
                b, s0 = bs(k)
                osb = osp.tile([128, 4, 512], F32, tag="osb")
                for c in range(4):
                    po = pbig.tile([128, 512], F32, tag="big")
                    for j in range(4):
                        nc.tensor.matmul(
                            out=po,
                            lhsT=attnT[:, c, j, :],
                            rhs=wo_sb[:, j, :],
                            start=(j == 0), stop=(j == 3),
                        )
                    nc.vector.tensor_add(osb[:, c, :], po, bias_b)
                    if last:
                        # drain-time tile: store chunk-wise on SP so the
                        # final transfers overlap the remaining adds
                        nc.sync.dma_start(
                            out=out_d[b, s0 + c * 128:s0 + (c + 1) * 128, :],
                            in_=osb[:, c, :],
                        )
                if not last:
                    nc.gpsimd.dma_start(
                        out=out_d[b, s0:s0 + 512, :].rearrange("(t p) d -> p t d", p=128),
                        in_=osb,
                    )

            # ---- software-pipelined main loop ----
            # st[k] = dict of live per-tile objects
            st = {}
            kv_of = {}  # tile index -> (kT_sb, v_aug)

            # prologue, in DMA service order: ctx(0) (tiny, kv-critical),
            # then Wk/Wv (kv matmuls), then x(0)/Wq (first qT), then the rest
            ctxT0 = emit_ctx(0)
            nc.gpsimd.dma_start(out=wk_sb, in_=wk_d[:].rearrange("(c p) e -> p c e", p=128))
            st[0] = {"x": emit_xload(0)}
            nc.gpsimd.dma_start(out=wq_sb, in_=wq_d[:].rearrange("(c p) e -> p c e", p=128))
            st[0]["xT"] = emit_xT(st[0]["x"])
            nc.gpsimd.dma_start(out=wv_sb, in_=wv_d[:].rearrange("(c p) e -> p c e", p=128))
            kv = emit_kv(ctxT0)
            st[1] = {"x": emit_xload(1)}
            nc.gpsimd.dma_start(out=wo_sb, in_=wo_d[:].rearrange("(c p) e -> p c e", p=128))
            nc.gpsimd.dma_start(out=bias_b, in_=bo_d[:].partition_broadcast(128))

            for k in range(NTILES + 2):
                # stage A: next-next x load
                if k + 2 < NTILES:
                    st[k + 2] = {"x": emit_xload(k + 2)}
                # stage B: next xT transpose
                if 0 < k + 1 < NTILES:
                    st[k + 1]["xT"] = emit_xT(st[k + 1]["x"])

                if k < NTILES:
                    b = k // NT
                    # prefetch next batch's ctx early and its k/v mid-batch so
                    # the PE work and ctx DMA land before the batch boundary
                    if k % NT == 2 and b + 1 < BPC:
                        ctxT_next = emit_ctx(b + 1)
                    if k % NT == NT - 3 and b + 1 < BPC:
                        kv_next = emit_kv(ctxT_next)
                    if k % NT == 0 and k > 0:
                        kv = kv_next
                    kv_of[k] = kv
                    # PE stage 1: qT(k). At k=0 the DVE queue is empty while
                    # ACT churns kv copies + first exps — use all-DVE copies
                    if "qT" not in st[k]:
                        st[k]["qT"] = emit_qT(st[k]["xT"], all_dve=(k == 0))
                    # PE stage 2: scores+exp(k)
                    st[k]["et"] = emit_scores(st[k]["qT"], kv_of[k][0])
                # PE stage 3: attnU/norm/transpose(k-1)
                if 0 <= k - 1 < NTILES and "attnT" not in st[k - 1]:
                    st[k - 1]["attnT"] = emit_attn(st[k - 1]["et"], kv_of[k - 1][1])
                # PE stage 4: outproj(k-2)
                if 0 <= k - 2:
                    emit_outproj(st[k - 2]["attnT"], k - 2, last=(k - 2 >= NTILES - 2))
                    del st[k - 2]
                # epilogue shortcut: run the last tile's attention stage
                # lag-0 (its exps are long done by this point in the PE
                # stream) so the final outproj isn't stuck behind a fresh
                # DMA transpose at drain time
                if k == NTILES - 1:
                    st[k]["attnT"] = emit_attn(st[k]["et"], kv_of[k][1])

    # TRN2 hardware allows at most 1 semaphore wait per instruction; split
    # multi-wait instructions into standalone EventSemaphore waits.
    _bass_rust.generate_event_semaphores(nc)
    return nc


_NC_CACHE = None


def kernel(x, context, Wq, Wk, Wv, Wout, bout):
    global _NC_CACHE
    if _NC_CACHE is None:
        _NC_CACHE = build_nc()
    nc = _NC_CACHE

    f = lambda a: np.ascontiguousarray(np.asarray(a), dtype=np.float32)
    x, context = f(x), f(context)
    Wq, Wk, Wv, Wout, bout = f(Wq), f(Wk), f(Wv), f(Wout), f(bout)

    in_maps = [
        {
            "x": x[c * BPC:(c + 1) * BPC],
            "context": context[c * BPC:(c + 1) * BPC],
            "Wq": Wq, "Wk": Wk, "Wv": Wv, "Wout": Wout, "bout": bout,
        }
        for c in range(N_CORES)
    ]
    res = run_bass_kernel_spmd(nc, in_maps, core_ids=list(range(N_CORES)))
    return np.concatenate([r["out"] for r in res.results], axis=0)



# revision 5
# speedup vs baseline: 1.0268x; 1.0268x over previous
"""Cross-attention Trainium2 Bass kernel (bf16 + compensated-fp8, pipelined).

Sharding: data-parallel over batch — 16 batches across 8 cores, 2 per core.
Weights replicated. Each core computes its 2 batches fully; no collectives.

The Q projection runs as error-compensated fp8 DoubleRow matmuls:
  q = x8hi @ (256*Wq)8hi + x8hi @ (256*Wq)8lo + x8lo @ (256*Wq)8hi   (/256)
Each term is an fp8e4m3 DoubleRow matmul (0.5 PE cycles/row, 2 contraction
slabs of 128 per instruction), so K=512 takes 6 DR instructions per output
chunk-slice instead of 4 bf16 ones: 6144 vs 8192 PE cycles per x tile.
Dropping the lo*lo term leaves ~1e-3 relative error (measured), well inside
the 2e-2 gate. Scores / attention / output projection stay bf16: for K<=128
the 2-slab DoubleRow format makes compensated fp8 cost the same as bf16.

PE transposes are eliminated entirely: x and attn are transposed by the
DMA crossbar (dma_start_transpose, 2-byte dtype, 16x128 xbar tiles), whose
destination mapping is out[p, c, f] = in[f, c*128 + p] (verified on hw).

Per 512-row x tile:
  x_bf   = bf16(x tile)                 (SWDGE casting DMA, Pool engine)
  xT     = DMA-transpose(x_bf)          -> [d, s] layout
  x8h    = fp8e4(xT), x8l = fp8e4(xT - x8h)    (GPSIMD copy + subtract)
  qT     = comp-fp8 DR matmuls          (PE; PSUM->SBUF copy on ACT with
                                         the 1/256 descale folded in)
  per head h:  scT = kT_h^T @ qT_h      [77, 512] (PE, bf16)
               et_h = exp(0.125 * scT)  (ACT, bf16 out)
  per s-chunk c (128 rows): all 8 heads' attnU into one 2-bank PSUM tile
    pa[:, g, hh, 0:66] = et_h_chunk^T @ [v_h | 1 | 1]  (PE; col 64 = denom)
    rr = 1/pa[..., 64]                  (DVE, one recip per chunk)
    attn_n = pa[..., 0:64] * rr         (DVE, bf16, one mul per chunk)
  attnT  = DMA-transpose(attn_n)        -> [e, s] layout
  out    = attnT^T @ Wout + bout        (PE bf16; bias added during the
           PSUM->SBUF move by DVE tensor_add; HWDGE store on SP)

Engine budget per tile (ns, cost-model): PE 8560, ACT ~7400 (exp + qT
copies), DVE ~6600 (attn norm + bias), Pool ~7100 (SWDGE x load + fp8
hi/lo split), DMA ~8000. The per-engine streams are software-pipelined
with a 2-tile lag as in the bf16 baseline.

Weights are cast-loaded fp32->bf16 by SWDGE DMA (Wk/Wv/Wout/bias), laid
out "(c p) e -> p c e" so partition p of chunk c holds row c*128+p,
matching the DMA-transpose output mapping. Wq is loaded fp32 and split
hi/lo on ACT+DVE in the prologue. TRN2 allows 1 semaphore wait per
instruction — generate_event_semaphores() legalizes multi-wait
instructions that Tile emits.
"""

import numpy as np

import bass_rust as _bass_rust
import concourse.bass as bass
import concourse.mybir as mybir
import concourse.tile as tile
from concourse.bass import broadcast_tensor_aps
from concourse.bass_utils import run_bass_kernel_spmd

N_CORES = 8
B, SQ, DM = 16, 4096, 512
SKV, DC = 77, 768
H, DH = 8, 64
INNER = 512
BPC = B // N_CORES  # batches per core
NT = SQ // 512      # x tiles per batch
NTILES = BPC * NT   # total x tiles per core

F32 = mybir.dt.float32
BF16 = mybir.dt.bfloat16
E4 = mybir.dt.float8e4

AF = mybir.ActivationFunctionType
PM = mybir.MatmulPerfMode

WQS = 256.0  # Wq pre-scale so its fp8 split stays out of subnormals


def build_nc(trace_sim=False, nbig=2, nsc=2, nau=2, nqt=2, net=2, nan=2, nat=2, nos=2, nkv=2, nxl=3, nx8=2):
    nc = bass.Bass()

    x_d = nc.dram_tensor("x", [BPC, SQ, DM], F32, kind="ExternalInput")
    ctx_d = nc.dram_tensor("context", [BPC, SKV, DC], F32, kind="ExternalInput")
    wq_d = nc.dram_tensor("Wq", [DM, INNER], F32, kind="ExternalInput")
    wk_d = nc.dram_tensor("Wk", [DC, INNER], F32, kind="ExternalInput")
    wv_d = nc.dram_tensor("Wv", [DC, INNER], F32, kind="ExternalInput")
    wo_d = nc.dram_tensor("Wout", [INNER, INNER], F32, kind="ExternalInput")
    bo_d = nc.dram_tensor("bout", [INNER], F32, kind="ExternalInput")
    out_d = nc.dram_tensor("out", [BPC, SQ, DM], F32, kind="ExternalOutput")

    with tile.TileContext(nc, trace_sim=trace_sim) as tc:
        with (
            tc.tile_pool(name="const", bufs=1) as consts,
            tc.tile_pool(name="kvp", bufs=nkv) as kvp,
            tc.tile_pool(name="xload", bufs=nxl) as xload,
            tc.tile_pool(name="xtp", bufs=2) as xtp,
            tc.tile_pool(name="x8p", bufs=nx8) as x8p,
            tc.tile_pool(name="qtp", bufs=nqt) as qtp,
            tc.tile_pool(name="etp", bufs=net) as etp,
            tc.tile_pool(name="rrp", bufs=8) as rrp,
            tc.tile_pool(name="anp", bufs=nan) as anp,
            tc.tile_pool(name="atp", bufs=nat) as atp,
            tc.tile_pool(name="osp", bufs=nos) as osp,
            tc.tile_pool(name="pbig", bufs=nbig, space="PSUM") as pbig,
            tc.tile_pool(name="psc", bufs=nsc, space="PSUM") as psc,
            tc.tile_pool(name="pau", bufs=nau, space="PSUM") as pau,
        ):
            # ---- weights ----
            # layout "(c p) e -> p c e": partition p of chunk c holds row
            # c*128+p — same mapping as the DMA-transpose destination.
            wk_sb = consts.tile([128, DC // 128, INNER], BF16, tag="wk")
            wv_sb = consts.tile([128, DC // 128, INNER], BF16, tag="wv")
            wo_sb = consts.tile([128, INNER // 128, INNER], BF16, tag="wo")
            bias_b = consts.tile([128, INNER], F32, tag="bias")
            wqf = consts.tile([128, DM // 128, INNER], F32, tag="wqf")
            wq8h = consts.tile([128, DM // 128, INNER], E4, tag="wq8h")
            wq8l = consts.tile([128, DM // 128, INNER], E4, tag="wq8l")

            def emit_wq_split():
                # hi on ACT (scaled x256), lo on DVE; both fp8e4
                nc.scalar.activation(out=wq8h, in_=wqf, func=AF.Copy, scale=WQS)
                nc.vector.scalar_tensor_tensor(
                    wq8l, wqf, WQS, wq8h,
                    mybir.AluOpType.mult, mybir.AluOpType.subtract,
                )

            def emit_ctx(b):
                # ctx cast-load into a 80-partition tile (pad rows 77..79 are
                # never read downstream; DMA-T needs p % 16 == 0)
                ctx_bf = kvp.tile([80, DC], BF16, tag="ctx")
                nc.gpsimd.dma_start(out=ctx_bf[0:SKV, :], in_=ctx_d[b])
                ctxT = kvp.tile([128, DC // 128, 80], BF16, tag="ctxT")
                nc.sync.dma_start_transpose(out=ctxT, in_=ctx_bf[:, :])
                return ctxT

            def emit_kT(ctxT):
                # kT[e, kv]: lhsT = Wk chunk, rhs = ctxT chunk
                kT_sb = kvp.tile([128, INNER // 128, SKV], BF16, tag="kT")
                for i in range(INNER // 128):
                    pk = pbig.tile([128, 512], F32, tag="big")
                    for j in range(DC // 128):
                        nc.tensor.matmul(
                            out=pk[:, 0:SKV],
                            lhsT=wk_sb[:, j, i * 128:(i + 1) * 128],
                            rhs=ctxT[:, j, 0:SKV],
                            start=(j == 0), stop=(j == DC // 128 - 1),
                        )
                    nc.vector.tensor_copy(kT_sb[:, i, :], pk[:, 0:SKV])
                return kT_sb

            def emit_v(ctxT):
                # v computed transposed ([e, kv], full-width matmuls), then
                # DMA-transposed back to [kv, e]; only kv rows 0..76 of the
                # transpose output are ever read, so pad columns stay garbage
                vT_sb = kvp.tile([128, INNER // 128, 128], BF16, tag="vT")
                for i in range(INNER // 128):
                    pv = pbig.tile([128, 512], F32, tag="big")
                    for j in range(DC // 128):
                        nc.tensor.matmul(
                            out=pv[:, 0:SKV],
                            lhsT=wv_sb[:, j, i * 128:(i + 1) * 128],
                            rhs=ctxT[:, j, 0:SKV],
                            start=(j == 0), stop=(j == DC // 128 - 1),
                        )
                    nc.vector.tensor_copy(vT_sb[:, i, 0:SKV], pv[:, 0:SKV])
                v_kv = kvp.tile([128, INNER // 128, 128], BF16, tag="v_kv")
                nc.sync.dma_start_transpose(out=v_kv, in_=vT_sb[:, :, :])

                # v_aug[kv, h, 0:64] = v_h, col 64 = 1 (softmax denominator)
                v_aug = kvp.tile([SKV, H, 66], BF16, tag="v_aug")
                nc.vector.tensor_copy(
                    v_aug[:, :, 0:64],
                    v_kv[0:SKV, :, :].rearrange("p a b -> p (a b)").rearrange(
                        "p (h d) -> p h d", h=H
                    ),
                )
                nc.vector.memset(v_aug[:, :, 64:66], 1.0)
                return v_aug

            def emit_kv(ctxT):
                kT_sb = emit_kT(ctxT)
                v_aug = emit_v(ctxT)
                return kT_sb, v_aug

            # per-tile stage emitters; state[k] carries live tiles of tile k
            def bs(k):
                return k // NT, (k % NT) * 512

            def emit_xload(k):
                b, s0 = bs(k)
                x_bf = xload.tile([128, 4, DM], BF16, tag="x")
                nc.gpsimd.dma_start(
                    out=x_bf,
                    in_=x_d[b, s0:s0 + 512, :].rearrange("(t p) d -> p t d", p=128),
                )
                return x_bf

            def emit_xT(x_bf):
                # one merged transpose: in [128, 2048] -> out row r = c*128+p
                # lands as xT[p, t, c, ss] = x[t*128+ss, c*128+p]
                xT = xtp.tile([128, 4, 4, 128], BF16, tag="xT")
                nc.sync.dma_start_transpose(out=xT, in_=x_bf[:, :, :])
                return xT

            def emit_x8(xT):
                # fp8 hi/lo split on GPSIMD (SBUF->SBUF, Pool engine).
                # Tiles are stored c-major ([p, c, t, ss], c = dm chunk) so a
                # DoubleRow rhs [p, 2, 512] has its 512 seq columns contiguous;
                # the writes land strided by permuting the out AP (free cost).
                x8h = x8p.tile([128, 4, 4, 128], E4, tag="x8h")
                x8l = x8p.tile([128, 4, 4, 128], E4, tag="x8l")
                nc.gpsimd.tensor_copy(x8h.rearrange("p c t s -> p t c s"), xT)
                nc.gpsimd.tensor_tensor(
                    x8l.rearrange("p c t s -> p t c s"), xT,
                    x8h.rearrange("p c t s -> p t c s"), mybir.AluOpType.subtract,
                )
                return x8h, x8l

            def emit_qT(x8):
                # compensated fp8 DoubleRow: per i-chunk, 6 full-width DR
                # matmuls (3 terms x 2 chunk-pairs), each contracting 2x128
                # dm rows over all 512 seq columns (256 PE cycles each).
                x8h, x8l = x8
                qT = qtp.tile([128, 4, 512], BF16, tag="qT")
                terms = [(wq8h, x8h), (wq8l, x8h), (wq8h, x8l)]
                for i in range(4):
                    pq = pbig.tile([128, 512], F32, tag="big")
                    np_ = 0
                    for wsb, xsb in terms:
                        for j2 in range(2):
                            nc.tensor.matmul(
                                out=pq,
                                lhsT=wsb[:, 2 * j2:2 * j2 + 2, i * 128:(i + 1) * 128],
                                rhs=xsb[:, 2 * j2:2 * j2 + 2, :, :].rearrange(
                                    "p c t s -> p c (t s)"
                                ),
                                start=(np_ == 0), stop=(np_ == 5),
                                perf_mode=PM.DoubleRow,
                            )
                            np_ += 1
                    # PSUM->SBUF on ACT with the 1/256 descale folded in
                    nc.scalar.activation(
                        out=qT[:, i, :], in_=pq, func=AF.Copy, scale=1.0 / WQS,
                    )
                return qT

            def emit_scores(qT, kT_sb):
                et = etp.tile([SKV, H, 512], BF16, tag="et")
                for h in range(H):
                    i, r0 = h // 2, (h % 2) * 64
                    ps = psc.tile([SKV, 512], F32, tag="sc")
                    nc.tensor.matmul(
                        out=ps,
                        lhsT=kT_sb[r0:r0 + 64, i, :],
                        rhs=qT[r0:r0 + 64, i, :],
                        start=True, stop=True,
                    )
                    nc.scalar.activation(
                        out=et[:, h, :], in_=ps, func=AF.Exp, scale=0.125,
                    )
                return et

            def emit_attn(et, v_aug):
                attn_n = anp.tile([128, 4, 512], BF16, tag="attn_n")
                attnT = atp.tile([128, 4, 4, 128], BF16, tag="attnT")
                for c in range(4):
                    # all 8 heads of this chunk into one 2-bank PSUM tile
                    pa = pau.tile([128, 2, 4, 128], F32, tag="attnU")
                    for g in range(2):
                        for hh in range(4):
                            h = g * 4 + hh
                            nc.tensor.matmul(
                                out=pa[:, g, hh, 0:66],
                                lhsT=et[:, h, c * 128:(c + 1) * 128],
                                rhs=v_aug[:, h, :],
                                start=True, stop=True,
                            )
                    rr = rrp.tile([128, 2, 4, 1], F32, tag="rr")
                    nc.vector.reciprocal(out=rr, in_=pa[:, :, :, 64:65])
                    out_ap = attn_n[:, c, :].rearrange(
                        "p (g h d) -> p g h d", g=2, h=4
                    )
                    in0, in1 = broadcast_tensor_aps(pa[:, :, :, 0:64], rr)
                    nc.vector.tensor_mul(out_ap, in0, in1)
                    nc.sync.dma_start_transpose(
                        out=attnT[:, c, :, :], in_=attn_n[:, c, :],
                    )
                return attnT

            def emit_outproj(attnT, k, last=False):
                b, s0 = bs(k)
                osb = osp.tile([128, 4, 512], F32, tag="osb")
                for c in range(4):
                    po = pbig.tile([128, 512], F32, tag="big")
                    for j in range(4):
                        nc.tensor.matmul(
                            out=po,
                            lhsT=attnT[:, c, j, :],
                            rhs=wo_sb[:, j, :],
                            start=(j == 0), stop=(j == 3),
                        )
                    nc.vector.tensor_add(osb[:, c, :], po, bias_b)
                    if last:
                        # drain-time tile: store chunk-wise on SP so the
                        # final transfers overlap the remaining adds
                        nc.sync.dma_start(
                            out=out_d[b, s0 + c * 128:s0 + (c + 1) * 128, :],
                            in_=osb[:, c, :],
                        )
                if not last:
                    nc.sync.dma_start(
                        out=out_d[b, s0:s0 + 512, :].rearrange("(t p) d -> p t d", p=128),
                        in_=osb,
                    )

            # ---- software-pipelined main loop ----
            # st[k] = dict of live per-tile objects
            st = {}
            kv_of = {}  # tile index -> (kT_sb, v_aug)

            # prologue, in DMA service order: ctx(0) (tiny, kv-critical),
            # then Wk/Wv (kv matmuls), then x(0)/Wq (first qT), then the rest
            ctxT0 = emit_ctx(0)
            nc.gpsimd.dma_start(out=wk_sb, in_=wk_d[:].rearrange("(c p) e -> p c e", p=128))
            st[0] = {"x": emit_xload(0)}
            nc.sync.dma_start(out=wqf, in_=wq_d[:].rearrange("(c p) e -> p c e", p=128))
            st[0]["xT"] = emit_xT(st[0]["x"])
            nc.gpsimd.dma_start(out=wv_sb, in_=wv_d[:].rearrange("(c p) e -> p c e", p=128))
            emit_wq_split()
            st[0]["x8"] = emit_x8(st[0]["xT"])
            kv = emit_kv(ctxT0)
            st[1] = {"x": emit_xload(1)}
            nc.gpsimd.dma_start(out=wo_sb, in_=wo_d[:].rearrange("(c p) e -> p c e", p=128))
            nc.gpsimd.dma_start(out=bias_b, in_=bo_d[:].partition_broadcast(128))

            for k in range(NTILES + 2):
                # stage A: next-next x load
                if k + 2 < NTILES:
                    st[k + 2] = {"x": emit_xload(k + 2)}
                # stage B: next xT transpose + fp8 split
                if 0 < k + 1 < NTILES:
                    st[k + 1]["xT"] = emit_xT(st[k + 1]["x"])
                    st[k + 1]["x8"] = emit_x8(st[k + 1]["xT"])

                if k < NTILES:
                    b = k // NT
                    # prefetch next batch's ctx early and its k/v mid-batch so
                    # the PE work and ctx DMA land before the batch boundary
                    if k % NT == 2 and b + 1 < BPC:
                        ctxT_next = emit_ctx(b + 1)
                    if k % NT == NT - 3 and b + 1 < BPC:
                        kv_next = emit_kv(ctxT_next)
                    if k % NT == 0 and k > 0:
                        kv = kv_next
                    kv_of[k] = kv
                    # PE stage 1: qT(k)
                    if "qT" not in st[k]:
                        st[k]["qT"] = emit_qT(st[k]["x8"])
                    # PE stage 2: scores+exp(k)
                    st[k]["et"] = emit_scores(st[k]["qT"], kv_of[k][0])
                # PE stage 3: attnU/norm/transpose(k-1)
                if 0 <= k - 1 < NTILES and "attnT" not in st[k - 1]:
                    st[k - 1]["attnT"] = emit_attn(st[k - 1]["et"], kv_of[k - 1][1])
                # PE stage 4: outproj(k-2)
                if 0 <= k - 2:
                    emit_outproj(st[k - 2]["attnT"], k - 2, last=(k - 2 >= NTILES - 2))
                    del st[k - 2]
                # epilogue shortcut: run the last tile's attention stage
                # lag-0 (its exps are long done by this point in the PE
                # stream) so the final outproj isn't stuck behind a fresh
                # DMA transpose at drain time
                if k == NTILES - 1:
                    st[k]["attnT"] = emit_attn(st[k]["et"], kv_of[k][1])

    # TRN2 hardware allows at most 1 semaphore wait per instruction; split
    # multi-wait instructions into standalone EventSemaphore waits.
    _bass_rust.generate_event_semaphores(nc)
    return nc


_NC_CACHE = None


def kernel(x, context, Wq, Wk, Wv, Wout, bout):
    global _NC_CACHE
    if _NC_CACHE is None:
        _NC_CACHE = build_nc()
    nc = _NC_CACHE

    f = lambda a: np.ascontiguousarray(np.asarray(a), dtype=np.float32)
    x, context = f(x), f(context)
    Wq, Wk, Wv, Wout, bout = f(Wq), f(Wk), f(Wv), f(Wout), f(bout)

    in_maps = [
        {
            "x": x[c * BPC:(c + 1) * BPC],
            "context": context[c * BPC:(c + 1) * BPC],
            "Wq": Wq, "Wk": Wk, "Wv": Wv, "Wout": Wout, "bout": bout,
        }
        for c in range(N_CORES)
    ]
    res = run_bass_kernel_spmd(nc, in_maps, core_ids=list(range(N_CORES)))
    return np.concatenate([r["out"] for r in res.results], axis=0)


# revision 10
# speedup vs baseline: 1.0349x; 1.0079x over previous
"""Cross-attention Trainium2 Bass kernel (bf16 + compensated-fp8, pipelined).

Sharding: data-parallel over batch — 16 batches across 8 cores, 2 per core.
Weights replicated. Each core computes its 2 batches fully; no collectives.

The Q projection runs as error-compensated fp8 DoubleRow matmuls:
  q = x8hi @ (256*Wq)8hi + x8hi @ (256*Wq)8lo + x8lo @ (256*Wq)8hi   (/256)
Each term is an fp8e4m3 DoubleRow matmul (0.5 PE cycles/row, 2 contraction
slabs of 128 per instruction), so K=512 takes 6 DR instructions per output
chunk-slice instead of 4 bf16 ones: 6144 vs 8192 PE cycles per x tile.
Dropping the lo*lo term leaves ~1e-3 relative error (measured), well inside
the 2e-2 gate. Scores / attention / output projection stay bf16: for K<=128
the 2-slab DoubleRow format makes compensated fp8 cost the same as bf16.

PE transposes are eliminated entirely: x and attn are transposed by the
DMA crossbar (dma_start_transpose, 2-byte dtype, 16x128 xbar tiles), whose
destination mapping is out[p, c, f] = in[f, c*128 + p] (verified on hw).

Per 512-row x tile:
  x_bf   = bf16(x tile)                 (SWDGE casting DMA, Pool engine)
  xT     = DMA-transpose(x_bf)          -> [d, s] layout
  x8h    = fp8e4(xT), x8l = fp8e4(xT - x8h)    (GPSIMD copy + subtract)
  qT     = comp-fp8 DR matmuls          (PE; PSUM->SBUF copy on ACT with
                                         the 1/256 descale folded in)
  per head h:  scT = kT_h^T @ qT_h      [77, 512] (PE, bf16)
               et_h = exp(0.125 * scT)  (ACT, bf16 out)
  per s-chunk c (128 rows): all 8 heads' attnU into one 2-bank PSUM tile
    pa[:, g, hh, 0:66] = et_h_chunk^T @ [v_h | 1 | 1]  (PE; col 64 = denom)
    rr = 1/pa[..., 64]                  (DVE, one recip per chunk)
    attn_n = pa[..., 0:64] * rr         (DVE, bf16, one mul per chunk)
  attnT  = DMA-transpose(attn_n)        -> [e, s] layout
  out    = attnT^T @ Wout + bout        (PE bf16; bias added during the
           PSUM->SBUF move by DVE tensor_add; HWDGE store on SP)

Engine budget per tile (ns, cost-model): PE 8560, ACT ~7400 (exp + qT
copies), DVE ~6600 (attn norm + bias), Pool ~7100 (SWDGE x load + fp8
hi/lo split), DMA ~8000. The per-engine streams are software-pipelined
with a 2-tile lag as in the bf16 baseline.

Weights are cast-loaded fp32->bf16 by SWDGE DMA (Wk/Wv/Wout/bias), laid
out "(c p) e -> p c e" so partition p of chunk c holds row c*128+p,
matching the DMA-transpose output mapping. Wq is loaded fp32 and split
hi/lo on ACT+DVE in the prologue. TRN2 allows 1 semaphore wait per
instruction — generate_event_semaphores() legalizes multi-wait
instructions that Tile emits.
"""

import numpy as np

import bass_rust as _bass_rust
import concourse.bass as bass
import concourse.mybir as mybir
import concourse.tile as tile
from concourse.bass import broadcast_tensor_aps
from concourse.bass_utils import run_bass_kernel_spmd

N_CORES = 8
B, SQ, DM = 16, 4096, 512
SKV, DC = 77, 768
H, DH = 8, 64
INNER = 512
BPC = B // N_CORES  # batches per core
NT = SQ // 512      # x tiles per batch
NTILES = BPC * NT   # total x tiles per core

F32 = mybir.dt.float32
BF16 = mybir.dt.bfloat16
E4 = mybir.dt.float8e4

AF = mybir.ActivationFunctionType
PM = mybir.MatmulPerfMode

WQS = 256.0  # Wq pre-scale so its fp8 split stays out of subnormals


def build_nc(trace_sim=False, nbig=3, nsc=3, nau=2, nqt=2, net=2, nan=2, nat=2, nos=2, nkv=2, nxl=3, nx8=2):
    nc = bass.Bass()

    x_d = nc.dram_tensor("x", [BPC, SQ, DM], F32, kind="ExternalInput")
    ctx_d = nc.dram_tensor("context", [BPC, SKV, DC], F32, kind="ExternalInput")
    wq_d = nc.dram_tensor("Wq", [DM, INNER], F32, kind="ExternalInput")
    wk_d = nc.dram_tensor("Wk", [DC, INNER], F32, kind="ExternalInput")
    wv_d = nc.dram_tensor("Wv", [DC, INNER], F32, kind="ExternalInput")
    wo_d = nc.dram_tensor("Wout", [INNER, INNER], F32, kind="ExternalInput")
    bo_d = nc.dram_tensor("bout", [INNER], F32, kind="ExternalInput")
    out_d = nc.dram_tensor("out", [BPC, SQ, DM], F32, kind="ExternalOutput")

    with tile.TileContext(nc, trace_sim=trace_sim) as tc:
        with (
            tc.tile_pool(name="const", bufs=1) as consts,
            tc.tile_pool(name="kvp", bufs=nkv) as kvp,
            tc.tile_pool(name="xload", bufs=nxl) as xload,
            tc.tile_pool(name="xtp", bufs=2) as xtp,
            tc.tile_pool(name="x8p", bufs=nx8) as x8p,
            tc.tile_pool(name="qtp", bufs=nqt) as qtp,
            tc.tile_pool(name="etp", bufs=net) as etp,
            tc.tile_pool(name="rrp", bufs=8) as rrp,
            tc.tile_pool(name="anp", bufs=nan) as anp,
            tc.tile_pool(name="atp", bufs=nat) as atp,
            tc.tile_pool(name="osp", bufs=nos) as osp,
            tc.tile_pool(name="pbig", bufs=nbig, space="PSUM") as pbig,
            tc.tile_pool(name="psc", bufs=nsc, space="PSUM") as psc,
            tc.tile_pool(name="pau", bufs=nau, space="PSUM") as pau,
        ):
            # ---- weights ----
            # layout "(c p) e -> p c e": partition p of chunk c holds row
            # c*128+p — same mapping as the DMA-transpose destination.
            wk_sb = consts.tile([128, DC // 128, INNER], BF16, tag="wk")
            wv_sb = consts.tile([128, DC // 128, INNER], BF16, tag="wv")
            wo_sb = consts.tile([128, INNER // 128, INNER], BF16, tag="wo")
            bias_b = consts.tile([128, INNER], F32, tag="bias")
            wq_bf = consts.tile([128, DM // 128, INNER], BF16, tag="wq_bf")
            wqf = consts.tile([128, DM // 128, INNER], F32, tag="wqf")
            wq8h = consts.tile([128, DM // 128, INNER], E4, tag="wq8h")
            wq8l = consts.tile([128, DM // 128, INNER], E4, tag="wq8l")

            def emit_wq_split():
                # hi on ACT (scaled x256), lo on DVE; both fp8e4
                nc.scalar.activation(out=wq8h, in_=wqf, func=AF.Copy, scale=WQS)
                nc.vector.scalar_tensor_tensor(
                    wq8l, wqf, WQS, wq8h,
                    mybir.AluOpType.mult, mybir.AluOpType.subtract,
                )

            def emit_ctx(b):
                # ctx cast-load into a 80-partition tile (pad rows 77..79 are
                # never read downstream; DMA-T needs p % 16 == 0)
                ctx_bf = kvp.tile([80, DC], BF16, tag="ctx")
                nc.gpsimd.dma_start(out=ctx_bf[0:SKV, :], in_=ctx_d[b])
                ctxT = kvp.tile([128, DC // 128, 80], BF16, tag="ctxT")
                nc.sync.dma_start_transpose(out=ctxT, in_=ctx_bf[:, :])
                return ctxT

            def emit_kT(ctxT):
                # kT[e, kv]: lhsT = Wk chunk, rhs = ctxT chunk
                kT_sb = kvp.tile([128, INNER // 128, SKV], BF16, tag="kT")
                for i in range(INNER // 128):
                    pk = pbig.tile([128, 512], F32, tag="big")
                    for j in range(DC // 128):
                        nc.tensor.matmul(
                            out=pk[:, 0:SKV],
                            lhsT=wk_sb[:, j, i * 128:(i + 1) * 128],
                            rhs=ctxT[:, j, 0:SKV],
                            start=(j == 0), stop=(j == DC // 128 - 1),
                        )
                    nc.vector.tensor_copy(kT_sb[:, i, :], pk[:, 0:SKV])
                return kT_sb

            def emit_v(ctxT):
                # v computed transposed ([e, kv], full-width matmuls), then
                # DMA-transposed back to [kv, e]; only kv rows 0..76 of the
                # transpose output are ever read, so pad columns stay garbage
                vT_sb = kvp.tile([128, INNER // 128, 128], BF16, tag="vT")
                for i in range(INNER // 128):
                    pv = pbig.tile([128, 512], F32, tag="big")
                    for j in range(DC // 128):
                        nc.tensor.matmul(
                            out=pv[:, 0:SKV],
                            lhsT=wv_sb[:, j, i * 128:(i + 1) * 128],
                            rhs=ctxT[:, j, 0:SKV],
                            start=(j == 0), stop=(j == DC // 128 - 1),
                        )
                    nc.vector.tensor_copy(vT_sb[:, i, 0:SKV], pv[:, 0:SKV])
                v_kv = kvp.tile([128, INNER // 128, 128], BF16, tag="v_kv")
                nc.sync.dma_start_transpose(out=v_kv, in_=vT_sb[:, :, :])

                # v_aug[kv, h, 0:64] = v_h, col 64 = 1 (softmax denominator)
                v_aug = kvp.tile([SKV, H, 66], BF16, tag="v_aug")
                nc.vector.tensor_copy(
                    v_aug[:, :, 0:64],
                    v_kv[0:SKV, :, :].rearrange("p a b -> p (a b)").rearrange(
                        "p (h d) -> p h d", h=H
                    ),
                )
                nc.vector.memset(v_aug[:, :, 64:66], 1.0)
                return v_aug

            def emit_kv(ctxT):
                kT_sb = emit_kT(ctxT)
                v_aug = emit_v(ctxT)
                return kT_sb, v_aug

            # per-tile stage emitters; state[k] carries live tiles of tile k
            def bs(k):
                return k // NT, (k % NT) * 512

            def emit_xload(k):
                b, s0 = bs(k)
                x_bf = xload.tile([128, 4, DM], BF16, tag="x")
                nc.gpsimd.dma_start(
                    out=x_bf,
                    in_=x_d[b, s0:s0 + 512, :].rearrange("(t p) d -> p t d", p=128),
                )
                return x_bf

            def emit_xT(x_bf):
                # one merged transpose: in [128, 2048] -> out row r = c*128+p
                # lands as xT[p, t, c, ss] = x[t*128+ss, c*128+p]
                xT = xtp.tile([128, 4, 4, 128], BF16, tag="xT")
                nc.sync.dma_start_transpose(out=xT, in_=x_bf[:, :, :])
                return xT

            def emit_x8(xT):
                # fp8 hi/lo split on GPSIMD (SBUF->SBUF, Pool engine).
                # Tiles are stored c-major ([p, c, t, ss], c = dm chunk) so a
                # DoubleRow rhs [p, 2, 512] has its 512 seq columns contiguous;
                # the writes land strided by permuting the out AP (free cost).
                x8h = x8p.tile([128, 4, 4, 128], E4, tag="x8h")
                x8l = x8p.tile([128, 4, 4, 128], E4, tag="x8l")
                nc.gpsimd.tensor_copy(x8h.rearrange("p c t s -> p t c s"), xT)
                nc.gpsimd.tensor_tensor(
                    x8l.rearrange("p c t s -> p t c s"), xT,
                    x8h.rearrange("p c t s -> p t c s"), mybir.AluOpType.subtract,
                )
                return x8h, x8l

            def emit_qT(x8):
                # compensated fp8 DoubleRow: per i-chunk, 6 full-width DR
                # matmuls (3 terms x 2 chunk-pairs), each contracting 2x128
                # dm rows over all 512 seq columns (256 PE cycles each).
                x8h, x8l = x8
                qT = qtp.tile([128, 4, 512], BF16, tag="qT")
                terms = [(wq8h, x8h), (wq8l, x8h), (wq8h, x8l)]
                for i in range(4):
                    pq = pbig.tile([128, 512], F32, tag="big")
                    np_ = 0
                    for wsb, xsb in terms:
                        for j2 in range(2):
                            nc.tensor.matmul(
                                out=pq,
                                lhsT=wsb[:, 2 * j2:2 * j2 + 2, i * 128:(i + 1) * 128],
                                rhs=xsb[:, 2 * j2:2 * j2 + 2, :, :].rearrange(
                                    "p c t s -> p c (t s)"
                                ),
                                start=(np_ == 0), stop=(np_ == 5),
                                perf_mode=PM.DoubleRow,
                            )
                            np_ += 1
                    # PSUM->SBUF on ACT with the 1/256 descale folded in
                    nc.scalar.activation(
                        out=qT[:, i, :], in_=pq, func=AF.Copy, scale=1.0 / WQS,
                    )
                return qT

            def emit_qT_bf16(xT):
                # plain bf16 q-projection from xT — used for tile 0 so the
                # first tile doesn't wait on the Wq fp8 hi/lo prep chain
                qT = qtp.tile([128, 4, 512], BF16, tag="qT")
                for i in range(4):
                    pq = pbig.tile([128, 512], F32, tag="big")
                    for j in range(4):
                        nc.tensor.matmul(
                            out=pq,
                            lhsT=wq_bf[:, j, i * 128:(i + 1) * 128],
                            rhs=xT[:, :, j, :],
                            start=(j == 0), stop=(j == 3),
                        )
                    nc.scalar.activation(out=qT[:, i, :], in_=pq, func=AF.Copy)
                return qT

            def emit_scores(qT, kT_sb):
                et = etp.tile([SKV, H, 512], BF16, tag="et")
                for h in range(H):
                    i, r0 = h // 2, (h % 2) * 64
                    ps = psc.tile([SKV, 512], F32, tag="sc")
                    nc.tensor.matmul(
                        out=ps,
                        lhsT=kT_sb[r0:r0 + 64, i, :],
                        rhs=qT[r0:r0 + 64, i, :],
                        start=True, stop=True,
                    )
                    nc.scalar.activation(
                        out=et[:, h, :], in_=ps, func=AF.Exp, scale=0.125,
                    )
                return et

            def emit_attn(et, v_aug):
                attn_n = anp.tile([128, 4, 512], BF16, tag="attn_n")
                attnT = atp.tile([128, 4, 4, 128], BF16, tag="attnT")
                for c in range(4):
                    for g in range(2):
                        pa = pau.tile([128, 4, 66], F32, tag="attnU")
                        for hh in range(4):
                            h = g * 4 + hh
                            nc.tensor.matmul(
                                out=pa[:, hh, :],
                                lhsT=et[:, h, c * 128:(c + 1) * 128],
                                rhs=v_aug[:, h, :],
                                start=True, stop=True,
                            )
                        rr = rrp.tile([128, 4, 1], F32, tag="rr")
                        nc.vector.reciprocal(out=rr, in_=pa[:, :, 64:65])
                        out_ap = attn_n[:, c, g * 256:(g + 1) * 256].rearrange(
                            "p (h d) -> p h d", h=4
                        )
                        in0, in1 = broadcast_tensor_aps(pa[:, :, 0:64], rr)
                        nc.vector.tensor_mul(out_ap, in0, in1)
                    nc.sync.dma_start_transpose(
                        out=attnT[:, c, :, :], in_=attn_n[:, c, :],
                    )
                return attnT

            def emit_outproj(attnT, k, last=False):
                b, s0 = bs(k)
                osb = osp.tile([128, 4, 512], F32, tag="osb")
                for c in range(4):
                    po = pbig.tile([128, 512], F32, tag="big")
                    for j in range(4):
                        nc.tensor.matmul(
                            out=po,
                            lhsT=attnT[:, c, j, :],
                            rhs=wo_sb[:, j, :],
                            start=(j == 0), stop=(j == 3),
                        )
                    nc.vector.tensor_add(osb[:, c, :], po, bias_b)
                    if last:
                        # drain-time tile: store chunk-wise on SP so the
                        # final transfers overlap the remaining adds
                        nc.sync.dma_start(
                            out=out_d[b, s0 + c * 128:s0 + (c + 1) * 128, :],
                            in_=osb[:, c, :],
                        )
                if not last:
                    nc.sync.dma_start(
                        out=out_d[b, s0:s0 + 512, :].rearrange("(t p) d -> p t d", p=128),
                        in_=osb,
                    )

            # ---- software-pipelined main loop ----
            # st[k] = dict of live per-tile objects
            st = {}
            kv_of = {}  # tile index -> (kT_sb, v_aug)

            # prologue, in DMA service order: ctx(0) (tiny, kv-critical),
            # then x(0)/Wq-bf16 (tile-0 qT), Wk/Wv (kv matmuls), then the
            # fp32 Wq for the fp8 split (feeds tile 1+), then the rest
            ctxT0 = emit_ctx(0)
            st[0] = {"x": emit_xload(0)}
            nc.gpsimd.dma_start(out=wq_bf, in_=wq_d[:].rearrange("(c p) e -> p c e", p=128))
            nc.gpsimd.dma_start(out=wk_sb, in_=wk_d[:].rearrange("(c p) e -> p c e", p=128))
            st[0]["xT"] = emit_xT(st[0]["x"])
            nc.gpsimd.dma_start(out=wv_sb, in_=wv_d[:].rearrange("(c p) e -> p c e", p=128))
            nc.sync.dma_start(out=wqf, in_=wq_d[:].rearrange("(c p) e -> p c e", p=128))
            st[0]["qT"] = emit_qT_bf16(st[0]["xT"])
            kv = emit_kv(ctxT0)
            emit_wq_split()
            st[1] = {"x": emit_xload(1)}
            nc.gpsimd.dma_start(out=wo_sb, in_=wo_d[:].rearrange("(c p) e -> p c e", p=128))
            nc.gpsimd.dma_start(out=bias_b, in_=bo_d[:].partition_broadcast(128))

            for k in range(NTILES + 2):
                # stage A: next-next x load
                if k + 2 < NTILES:
                    st[k + 2] = {"x": emit_xload(k + 2)}
                # stage B: next xT transpose + fp8 split
                if 0 < k + 1 < NTILES:
                    st[k + 1]["xT"] = emit_xT(st[k + 1]["x"])
                    st[k + 1]["x8"] = emit_x8(st[k + 1]["xT"])

                if k < NTILES:
                    b = k // NT
                    # prefetch next batch's ctx early and its k/v mid-batch so
                    # the PE work and ctx DMA land before the batch boundary
                    if k % NT == 2 and b + 1 < BPC:
                        ctxT_next = emit_ctx(b + 1)
                    if k % NT == NT - 3 and b + 1 < BPC:
                        kv_next = emit_kv(ctxT_next)
                    if k % NT == 0 and k > 0:
                        kv = kv_next
                    kv_of[k] = kv
                    # PE stage 1: qT(k)
                    if "qT" not in st[k]:
                        st[k]["qT"] = emit_qT(st[k]["x8"])
                    # PE stage 2: scores+exp(k)
                    st[k]["et"] = emit_scores(st[k]["qT"], kv_of[k][0])
                # PE stage 3: attnU/norm/transpose(k-1)
                if 0 <= k - 1 < NTILES and "attnT" not in st[k - 1]:
                    st[k - 1]["attnT"] = emit_attn(st[k - 1]["et"], kv_of[k - 1][1])
                # PE stage 4: outproj(k-2)
                if 0 <= k - 2:
                    emit_outproj(st[k - 2]["attnT"], k - 2, last=(k - 2 >= NTILES - 2))
                    del st[k - 2]
                # epilogue shortcut: run the last tile's attention stage
                # lag-0 (its exps are long done by this point in the PE
                # stream) so the final outproj isn't stuck behind a fresh
                # DMA transpose at drain time
                if k == NTILES - 1:
                    st[k]["attnT"] = emit_attn(st[k]["et"], kv_of[k][1])

    # TRN2 hardware allows at most 1 semaphore wait per instruction; split
    # multi-wait instructions into standalone EventSemaphore waits.
    _bass_rust.generate_event_semaphores(nc)
    return nc


_NC_CACHE = None


def kernel(x, context, Wq, Wk, Wv, Wout, bout):
    global _NC_CACHE
    if _NC_CACHE is None:
        _NC_CACHE = build_nc()
    nc = _NC_CACHE

    f = lambda a: np.ascontiguousarray(np.asarray(a), dtype=np.float32)
    x, context = f(x), f(context)
    Wq, Wk, Wv, Wout, bout = f(Wq), f(Wk), f(Wv), f(Wout), f(bout)

    in_maps = [
        {
            "x": x[c * BPC:(c + 1) * BPC],
            "context": context[c * BPC:(c + 1) * BPC],
            "Wq": Wq, "Wk": Wk, "Wv": Wv, "Wout": Wout, "bout": bout,
        }
        for c in range(N_CORES)
    ]
    res = run_bass_kernel_spmd(nc, in_maps, core_ids=list(range(N_CORES)))
    return np.concatenate([r["out"] for r in res.results], axis=0)


# revision 13
# speedup vs baseline: 1.0510x; 1.0155x over previous
"""Cross-attention Trainium2 Bass kernel (bf16 + compensated-fp8, pipelined).

Sharding: data-parallel over batch — 16 batches across 8 cores, 2 per core.
Weights replicated. Each core computes its 2 batches fully; no collectives.

The Q projection runs as error-compensated fp8 DoubleRow matmuls:
  q = x8hi @ (256*Wq)8hi + x8hi @ (256*Wq)8lo + x8lo @ (256*Wq)8hi   (/256)
Each term is an fp8e4m3 DoubleRow matmul (0.5 PE cycles/row, 2 contraction
slabs of 128 per instruction), so K=512 takes 6 DR instructions per output
chunk-slice instead of 4 bf16 ones: 6144 vs 8192 PE cycles per x tile.
Dropping the lo*lo term leaves ~1e-3 relative error (measured), well inside
the 2e-2 gate. Scores / attention / output projection stay bf16: for K<=128
the 2-slab DoubleRow format makes compensated fp8 cost the same as bf16.

PE transposes are eliminated entirely: x and attn are transposed by the
DMA crossbar (dma_start_transpose, 2-byte dtype, 16x128 xbar tiles), whose
destination mapping is out[p, c, f] = in[f, c*128 + p] (verified on hw).

Per 512-row x tile:
  x_bf   = bf16(x tile)                 (SWDGE casting DMA, Pool engine)
  xT     = DMA-transpose(x_bf)          -> [d, s] layout
  x8h    = fp8e4(xT), x8l = fp8e4(xT - x8h)    (GPSIMD copy + subtract)
  qT     = comp-fp8 DR matmuls          (PE; PSUM->SBUF copy on ACT with
                                         the 1/256 descale folded in)
  per head h:  scT = kT_h^T @ qT_h      [77, 512] (PE, bf16)
               et_h = exp(0.125 * scT)  (ACT, bf16 out)
  per s-chunk c (128 rows): all 8 heads' attnU into one 2-bank PSUM tile
    pa[:, g, hh, 0:66] = et_h_chunk^T @ [v_h | 1 | 1]  (PE; col 64 = denom)
    rr = 1/pa[..., 64]                  (DVE, one recip per chunk)
    attn_n = pa[..., 0:64] * rr         (DVE, bf16, one mul per chunk)
  attnT  = DMA-transpose(attn_n)        -> [e, s] layout
  out    = attnT^T @ Wout + bout        (PE bf16; bias added during the
           PSUM->SBUF move by DVE tensor_add; HWDGE store on SP)

Engine budget per tile (ns, cost-model): PE 8560, ACT ~7400 (exp + qT
copies), DVE ~6600 (attn norm + bias), Pool ~7100 (SWDGE x load + fp8
hi/lo split), DMA ~8000. The per-engine streams are software-pipelined
with a 2-tile lag as in the bf16 baseline.

Weights are cast-loaded fp32->bf16 by SWDGE DMA (Wk/Wv/Wout/bias), laid
out "(c p) e -> p c e" so partition p of chunk c holds row c*128+p,
matching the DMA-transpose output mapping. Wq is loaded fp32 and split
hi/lo on ACT+DVE in the prologue. TRN2 allows 1 semaphore wait per
instruction — generate_event_semaphores() legalizes multi-wait
instructions that Tile emits.
"""

import numpy as np

import bass_rust as _bass_rust
import concourse.bass as bass
import concourse.mybir as mybir
import concourse.tile as tile
from concourse.bass import broadcast_tensor_aps
from concourse.bass_utils import run_bass_kernel_spmd

N_CORES = 8
B, SQ, DM = 16, 4096, 512
SKV, DC = 77, 768
H, DH = 8, 64
INNER = 512
BPC = B // N_CORES  # batches per core
NT = SQ // 512      # x tiles per batch
NTILES = BPC * NT   # total x tiles per core

F32 = mybir.dt.float32
BF16 = mybir.dt.bfloat16
E4 = mybir.dt.float8e4

AF = mybir.ActivationFunctionType
PM = mybir.MatmulPerfMode

WQS = 256.0  # Wq pre-scale so its fp8 split stays out of subnormals


def build_nc(trace_sim=False, nbig=3, nsc=3, nau=2, nqt=2, net=2, nan=2, nat=2, nos=2, nkv=2, nxl=3, nx8=2):
    nc = bass.Bass()

    x_d = nc.dram_tensor("x", [BPC, SQ, DM], F32, kind="ExternalInput")
    ctx_d = nc.dram_tensor("context", [BPC, SKV, DC], F32, kind="ExternalInput")
    wq_d = nc.dram_tensor("Wq", [DM, INNER], F32, kind="ExternalInput")
    wk_d = nc.dram_tensor("Wk", [DC, INNER], F32, kind="ExternalInput")
    wv_d = nc.dram_tensor("Wv", [DC, INNER], F32, kind="ExternalInput")
    wo_d = nc.dram_tensor("Wout", [INNER, INNER], F32, kind="ExternalInput")
    bo_d = nc.dram_tensor("bout", [INNER], F32, kind="ExternalInput")
    out_d = nc.dram_tensor("out", [BPC, SQ, DM], F32, kind="ExternalOutput")

    with tile.TileContext(nc, trace_sim=trace_sim) as tc:
        with (
            tc.tile_pool(name="const", bufs=1) as consts,
            tc.tile_pool(name="kvp", bufs=nkv) as kvp,
            tc.tile_pool(name="xload", bufs=nxl) as xload,
            tc.tile_pool(name="xtp", bufs=2) as xtp,
            tc.tile_pool(name="x8p", bufs=nx8) as x8p,
            tc.tile_pool(name="qtp", bufs=nqt) as qtp,
            tc.tile_pool(name="etp", bufs=net) as etp,
            tc.tile_pool(name="rrp", bufs=8) as rrp,
            tc.tile_pool(name="anp", bufs=nan) as anp,
            tc.tile_pool(name="atp", bufs=nat) as atp,
            tc.tile_pool(name="osp", bufs=nos) as osp,
            tc.tile_pool(name="pbig", bufs=nbig, space="PSUM") as pbig,
            tc.tile_pool(name="psc", bufs=nsc, space="PSUM") as psc,
            tc.tile_pool(name="pau", bufs=nau, space="PSUM") as pau,
        ):
            # ---- weights ----
            # layout "(c p) e -> p c e": partition p of chunk c holds row
            # c*128+p — same mapping as the DMA-transpose destination.
            wk_sb = consts.tile([128, DC // 128, INNER], BF16, tag="wk")
            wv_sb = consts.tile([128, DC // 128, INNER], BF16, tag="wv")
            wo_sb = consts.tile([128, INNER // 128, INNER], BF16, tag="wo")
            bias_b = consts.tile([128, INNER], F32, tag="bias")
            wq_bf = consts.tile([128, DM // 128, INNER], BF16, tag="wq_bf")
            wqf = consts.tile([128, DM // 128, INNER], F32, tag="wqf")
            wq8h = consts.tile([128, DM // 128, INNER], E4, tag="wq8h")
            wq8l = consts.tile([128, DM // 128, INNER], E4, tag="wq8l")

            def emit_wq_split():
                # hi on ACT (scaled x256), lo on DVE; both fp8e4
                nc.scalar.activation(out=wq8h, in_=wqf, func=AF.Copy, scale=WQS)
                nc.vector.scalar_tensor_tensor(
                    wq8l, wqf, WQS, wq8h,
                    mybir.AluOpType.mult, mybir.AluOpType.subtract,
                )

            def emit_ctx(b):
                # ctx cast-load into a 80-partition tile (pad rows 77..79 are
                # never read downstream; DMA-T needs p % 16 == 0)
                ctx_bf = kvp.tile([80, DC], BF16, tag="ctx")
                nc.gpsimd.dma_start(out=ctx_bf[0:SKV, :], in_=ctx_d[b])
                ctxT = kvp.tile([128, DC // 128, 80], BF16, tag="ctxT")
                nc.sync.dma_start_transpose(out=ctxT, in_=ctx_bf[:, :])
                return ctxT

            def emit_kT(ctxT):
                # kT[e, kv]: lhsT = Wk chunk, rhs = ctxT chunk
                kT_sb = kvp.tile([128, INNER // 128, SKV], BF16, tag="kT")
                for i in range(INNER // 128):
                    pk = pbig.tile([128, 512], F32, tag="big")
                    for j in range(DC // 128):
                        nc.tensor.matmul(
                            out=pk[:, 0:SKV],
                            lhsT=wk_sb[:, j, i * 128:(i + 1) * 128],
                            rhs=ctxT[:, j, 0:SKV],
                            start=(j == 0), stop=(j == DC // 128 - 1),
                        )
                    nc.vector.tensor_copy(kT_sb[:, i, :], pk[:, 0:SKV])
                return kT_sb

            def emit_v(ctxT):
                # v computed transposed ([e, kv], full-width matmuls), then
                # DMA-transposed back to [kv, e]; only kv rows 0..76 of the
                # transpose output are ever read, so pad columns stay garbage
                vT_sb = kvp.tile([128, INNER // 128, 128], BF16, tag="vT")
                for i in range(INNER // 128):
                    pv = pbig.tile([128, 512], F32, tag="big")
                    for j in range(DC // 128):
                        nc.tensor.matmul(
                            out=pv[:, 0:SKV],
                            lhsT=wv_sb[:, j, i * 128:(i + 1) * 128],
                            rhs=ctxT[:, j, 0:SKV],
                            start=(j == 0), stop=(j == DC // 128 - 1),
                        )
                    nc.vector.tensor_copy(vT_sb[:, i, 0:SKV], pv[:, 0:SKV])
                v_kv = kvp.tile([128, INNER // 128, 128], BF16, tag="v_kv")
                nc.sync.dma_start_transpose(out=v_kv, in_=vT_sb[:, :, :])

                # v_aug[kv, h, 0:64] = v_h, col 64 = 1 (softmax denominator)
                v_aug = kvp.tile([SKV, H, 66], BF16, tag="v_aug")
                nc.vector.tensor_copy(
                    v_aug[:, :, 0:64],
                    v_kv[0:SKV, :, :].rearrange("p a b -> p (a b)").rearrange(
                        "p (h d) -> p h d", h=H
                    ),
                )
                nc.vector.memset(v_aug[:, :, 64:66], 1.0)
                return v_aug

            def emit_kv(ctxT):
                kT_sb = emit_kT(ctxT)
                v_aug = emit_v(ctxT)
                return kT_sb, v_aug

            # per-tile stage emitters; state[k] carries live tiles of tile k
            def bs(k):
                return k // NT, (k % NT) * 512

            def emit_xload(k):
                b, s0 = bs(k)
                x_bf = xload.tile([128, 4, DM], BF16, tag="x")
                nc.gpsimd.dma_start(
                    out=x_bf,
                    in_=x_d[b, s0:s0 + 512, :].rearrange("(t p) d -> p t d", p=128),
                )
                return x_bf

            def emit_xT(x_bf):
                # one merged transpose: in [128, 2048] -> out row r = c*128+p
                # lands as xT[p, t, c, ss] = x[t*128+ss, c*128+p]
                xT = xtp.tile([128, 4, 4, 128], BF16, tag="xT")
                nc.sync.dma_start_transpose(out=xT, in_=x_bf[:, :, :])
                return xT

            def emit_x8(xT):
                # fp8 hi/lo split on GPSIMD (SBUF->SBUF, Pool engine).
                # Tiles are stored c-major ([p, c, t, ss], c = dm chunk) so a
                # DoubleRow rhs [p, 2, 512] has its 512 seq columns contiguous;
                # the writes land strided by permuting the out AP (free cost).
                x8h = x8p.tile([128, 4, 4, 128], E4, tag="x8h")
                x8l = x8p.tile([128, 4, 4, 128], E4, tag="x8l")
                nc.gpsimd.tensor_copy(x8h.rearrange("p c t s -> p t c s"), xT)
                nc.gpsimd.tensor_tensor(
                    x8l.rearrange("p c t s -> p t c s"), xT,
                    x8h.rearrange("p c t s -> p t c s"), mybir.AluOpType.subtract,
                )
                return x8h, x8l

            def emit_qT(x8):
                # compensated fp8 DoubleRow: per i-chunk, 6 full-width DR
                # matmuls (3 terms x 2 chunk-pairs), each contracting 2x128
                # dm rows over all 512 seq columns (256 PE cycles each).
                x8h, x8l = x8
                qT = qtp.tile([128, 4, 512], BF16, tag="qT")
                terms = [(wq8h, x8h), (wq8l, x8h), (wq8h, x8l)]
                for i in range(4):
                    pq = pbig.tile([128, 512], F32, tag="big")
                    np_ = 0
                    for wsb, xsb in terms:
                        for j2 in range(2):
                            nc.tensor.matmul(
                                out=pq,
                                lhsT=wsb[:, 2 * j2:2 * j2 + 2, i * 128:(i + 1) * 128],
                                rhs=xsb[:, 2 * j2:2 * j2 + 2, :, :].rearrange(
                                    "p c t s -> p c (t s)"
                                ),
                                start=(np_ == 0), stop=(np_ == 5),
                                perf_mode=PM.DoubleRow,
                            )
                            np_ += 1
                    # PSUM->SBUF on ACT with the 1/256 descale folded in
                    nc.scalar.activation(
                        out=qT[:, i, :], in_=pq, func=AF.Copy, scale=1.0 / WQS,
                    )
                return qT

            def emit_qT_bf16(xT):
                # plain bf16 q-projection from xT — used for tile 0 so the
                # first tile doesn't wait on the Wq fp8 hi/lo prep chain
                qT = qtp.tile([128, 4, 512], BF16, tag="qT")
                for i in range(4):
                    pq = pbig.tile([128, 512], F32, tag="big")
                    for j in range(4):
                        nc.tensor.matmul(
                            out=pq,
                            lhsT=wq_bf[:, j, i * 128:(i + 1) * 128],
                            rhs=xT[:, :, j, :],
                            start=(j == 0), stop=(j == 3),
                        )
                    nc.scalar.activation(out=qT[:, i, :], in_=pq, func=AF.Copy)
                return qT

            def emit_scores(qT, kT_sb):
                et = etp.tile([SKV, H, 512], BF16, tag="et")
                for h in range(H):
                    i, r0 = h // 2, (h % 2) * 64
                    ps = psc.tile([SKV, 512], F32, tag="sc")
                    nc.tensor.matmul(
                        out=ps,
                        lhsT=kT_sb[r0:r0 + 64, i, :],
                        rhs=qT[r0:r0 + 64, i, :],
                        start=True, stop=True,
                    )
                    nc.scalar.activation(
                        out=et[:, h, :], in_=ps, func=AF.Exp, scale=0.125,
                    )
                return et

            def emit_attn(et, v_aug):
                attn_n = anp.tile([128, 4, 512], BF16, tag="attn_n")
                attnT = atp.tile([128, 4, 4, 128], BF16, tag="attnT")
                for c in range(4):
                    for g in range(2):
                        pa = pau.tile([128, 4, 66], F32, tag="attnU")
                        for hh in range(4):
                            h = g * 4 + hh
                            nc.tensor.matmul(
                                out=pa[:, hh, :],
                                lhsT=et[:, h, c * 128:(c + 1) * 128],
                                rhs=v_aug[:, h, :],
                                start=True, stop=True,
                            )
                        rr = rrp.tile([128, 4, 1], F32, tag="rr")
                        nc.vector.reciprocal(out=rr, in_=pa[:, :, 64:65])
                        out_ap = attn_n[:, c, g * 256:(g + 1) * 256].rearrange(
                            "p (h d) -> p h d", h=4
                        )
                        in0, in1 = broadcast_tensor_aps(pa[:, :, 0:64], rr)
                        nc.vector.tensor_mul(out_ap, in0, in1)
                    nc.sync.dma_start_transpose(
                        out=attnT[:, c, :, :], in_=attn_n[:, c, :],
                    )
                return attnT

            def emit_outproj(attnT, k, last=False):
                b, s0 = bs(k)
                osb = osp.tile([128, 4, 512], F32, tag="osb")
                for c in range(4):
                    po = pbig.tile([128, 512], F32, tag="big")
                    for j in range(4):
                        nc.tensor.matmul(
                            out=po,
                            lhsT=attnT[:, c, j, :],
                            rhs=wo_sb[:, j, :],
                            start=(j == 0), stop=(j == 3),
                        )
                    nc.vector.tensor_add(osb[:, c, :], po, bias_b)
                    if last:
                        # drain-time tile: store chunk-wise on SP so the
                        # final transfers overlap the remaining adds
                        nc.sync.dma_start(
                            out=out_d[b, s0 + c * 128:s0 + (c + 1) * 128, :],
                            in_=osb[:, c, :],
                        )
                if not last:
                    nc.sync.dma_start(
                        out=out_d[b, s0:s0 + 512, :].rearrange("(t p) d -> p t d", p=128),
                        in_=osb,
                    )

            # ---- software-pipelined main loop ----
            # st[k] = dict of live per-tile objects
            st = {}
            kv_of = {}  # tile index -> (kT_sb, v_aug)

            # prologue, in DMA service order: ctx(0) (tiny, kv-critical),
            # then x(0)/Wq-bf16 (tile-0 qT) and x(1)'s transpose + fp8 split
            # BEFORE the remaining weight loads, so tile 1's qT inputs don't
            # queue behind weight SWDGE descriptor generation on Pool
            ctxT0 = emit_ctx(0)
            st[0] = {"x": emit_xload(0)}
            nc.gpsimd.dma_start(out=wq_bf, in_=wq_d[:].rearrange("(c p) e -> p c e", p=128))
            nc.gpsimd.dma_start(out=wk_sb, in_=wk_d[:].rearrange("(c p) e -> p c e", p=128))
            st[0]["xT"] = emit_xT(st[0]["x"])
            st[1] = {"x": emit_xload(1)}
            nc.gpsimd.dma_start(out=wv_sb, in_=wv_d[:].rearrange("(c p) e -> p c e", p=128))
            nc.sync.dma_start(out=wqf, in_=wq_d[:].rearrange("(c p) e -> p c e", p=128))
            st[0]["qT"] = emit_qT_bf16(st[0]["xT"])
            st[1]["xT"] = emit_xT(st[1]["x"])
            st[1]["x8"] = emit_x8(st[1]["xT"])
            kv = emit_kv(ctxT0)
            emit_wq_split()
            st[2] = {"x": emit_xload(2)}
            nc.gpsimd.dma_start(out=wo_sb, in_=wo_d[:].rearrange("(c p) e -> p c e", p=128))
            nc.gpsimd.dma_start(out=bias_b, in_=bo_d[:].partition_broadcast(128))

            for k in range(NTILES + 2):
                # stage A: next-next x load
                if k + 2 < NTILES and k + 2 not in st:
                    st[k + 2] = {"x": emit_xload(k + 2)}
                # stage B: next xT transpose + fp8 split
                if 0 < k + 1 < NTILES and "xT" not in st[k + 1]:
                    st[k + 1]["xT"] = emit_xT(st[k + 1]["x"])
                    st[k + 1]["x8"] = emit_x8(st[k + 1]["xT"])

                if k < NTILES:
                    b = k // NT
                    # prefetch next batch's ctx early and its k/v mid-batch so
                    # the PE work and ctx DMA land before the batch boundary
                    if k % NT == 2 and b + 1 < BPC:
                        ctxT_next = emit_ctx(b + 1)
                    if k % NT == NT - 3 and b + 1 < BPC:
                        kv_next = emit_kv(ctxT_next)
                    if k % NT == 0 and k > 0:
                        kv = kv_next
                    kv_of[k] = kv
                    # PE stage 1: qT(k)
                    if "qT" not in st[k]:
                        st[k]["qT"] = emit_qT(st[k]["x8"])
                    # PE stage 2: scores+exp(k)
                    st[k]["et"] = emit_scores(st[k]["qT"], kv_of[k][0])
                # PE stage 3: attnU/norm/transpose(k-1)
                if 0 <= k - 1 < NTILES and "attnT" not in st[k - 1]:
                    st[k - 1]["attnT"] = emit_attn(st[k - 1]["et"], kv_of[k - 1][1])
                # epilogue shortcut: run the last tile's attention stage
                # lag-0, and BEFORE outproj(k-2) so its DMA transposes queue
                # ahead of the big output stores on the serialized DMA engines
                if k == NTILES - 1:
                    st[k]["attnT"] = emit_attn(st[k]["et"], kv_of[k][1])
                # PE stage 4: outproj(k-2)
                if 0 <= k - 2:
                    emit_outproj(st[k - 2]["attnT"], k - 2, last=(k - 2 >= NTILES - 3))
                    del st[k - 2]

    # TRN2 hardware allows at most 1 semaphore wait per instruction; split
    # multi-wait instructions into standalone EventSemaphore waits.
    _bass_rust.generate_event_semaphores(nc)
    return nc


_NC_CACHE = None


def kernel(x, context, Wq, Wk, Wv, Wout, bout):
    global _NC_CACHE
    if _NC_CACHE is None:
        _NC_CACHE = build_nc()
    nc = _NC_CACHE

    f = lambda a: np.ascontiguousarray(np.asarray(a), dtype=np.float32)
    x, context = f(x), f(context)
    Wq, Wk, Wv, Wout, bout = f(Wq), f(Wk), f(Wv), f(Wout), f(bout)

    in_maps = [
        {
            "x": x[c * BPC:(c + 1) * BPC],
            "context": context[c * BPC:(c + 1) * BPC],
            "Wq": Wq, "Wk": Wk, "Wv": Wv, "Wout": Wout, "bout": bout,
        }
        for c in range(N_CORES)
    ]
    res = run_bass_kernel_spmd(nc, in_maps, core_ids=list(range(N_CORES)))
    return np.concatenate([r["out"] for r in res.results], axis=0)
